# revision 1
# baseline (speedup 1.0000x reference)
"""Trainium2 Bass kernel for the neural-backflow problem.

Problem (hardcoded shapes): rs (4096, 3) f32 in a periodic box L=10.
For every electron pair (i, j): minimum-image displacement d_ij, distance
r_ij, force f_ij = MLP_spin(r_ij) (1->32->1 swish MLP with compact-support
decay; "same" weights for same-spin pairs, "diff" for cross-spin), output
rs + sum_j f_ij * d_ij.

Key algebraic reduction used here: with z_k = decay*w1_k + b1_k,
  force = decay^2 * sum_k (w1_k*wo_k) * sigmoid(z_k) + bo*decay
which is a smooth scalar function P(decay) on decay in (0, 1].  We fit a
degree-10 polynomial (Chebyshev fit, monomial coeffs, P(0)=0 forced) to P at
kernel-call time from the actual weight values, so the device program is
input-independent: the MLP collapses to a Horner chain of
scalar_tensor_tensor ops on the Vector engine.

decay itself is computed exactly (not approximated):
  m   = ((rs_j - rs_i + 15) mod 10) - 5          (= -minimum-image disp)
  r2  = m_x^2 + m_y^2 + m_z^2                     (matches sqrt(r2+1e-15)^2)
  g   = clamp(1 - 0.04*(r2 + 1e-15), >= 1-(1-1e-5)^2)   (= 1 - xn^2)
  decay = exp(1 - 1/g),  with 1/g = exp(-ln g) on the ACT engine
  (ScalarE Reciprocal is banned; Ln/Exp/Square/Copy share one ACT table set)

Sharding: rows of the pair grid across 8 cores (512 rows each); rs is
replicated (pre-broadcast across 128 partitions host-side for the j-axis
tiles).  Row-sums are local per core; outputs are concatenated.
"""

import numpy as np

import concourse.bass as bass
import concourse.mybir as mybir
from concourse.tile import TileContext
from concourse.bass_utils import run_bass_kernel_spmd

L = 10.0
N = 4096
N_UP = 2048
NCORES = 8
ROWS = N // NCORES          # 512 rows per core
JT = 512                    # j-tile width
NJT = N // JT               # 8 j-tiles
NIB = ROWS // 128           # 4 i-blocks of 128 rows per core
DEG = 10                    # polynomial degree
GMIN = float(np.float32(1.0) - np.float32((1.0 - 1e-5) ** 2))

F32 = mybir.dt.float32
AOP = mybir.AluOpType
AF = mybir.ActivationFunctionType

LAST_RESULTS = None  # BassKernelResults of the most recent run (for profiling)
_CACHED = {}         # built Bass program, keyed by nothing (shapes are fixed)


def _fit_poly(w1, b1, wo, bo):
    """Degree-DEG monomial coeffs of P(d) = d^2*S(d) + bo*d on d in [0,1],
    S(d) = sum_k w1_k*wo_k*sigmoid(w1_k*d + b1_k).  Returns c[1..DEG]
    (c[0] is forced to 0 exactly)."""
    w1 = np.asarray(w1, np.float64).ravel()
    b1 = np.asarray(b1, np.float64).ravel()
    wo = np.asarray(wo, np.float64).ravel()
    bo = float(np.asarray(bo, np.float64).ravel()[0])
    c = w1 * wo
    d = np.linspace(0.0, 1.0, 20001)
    z = d[:, None] * w1[None, :] + b1[None, :]
    S = (c[None, :] / (1.0 + np.exp(-z))).sum(axis=1)
    P = d * d * S + bo * d
    cheb = np.polynomial.chebyshev.Chebyshev.fit(d, P, DEG, domain=[0.0, 1.0])
    coef = cheb.convert(kind=np.polynomial.Polynomial).coef
    coef = np.resize(coef, DEG + 1)
    coef[0] = 0.0
    return coef[1:].astype(np.float32)  # c_1 .. c_DEG


def _build_program(reps=1):
    nc = bass.Bass()
    rsj = nc.declare_dram_parameter("rsj", [3, 128, N], F32, isOutput=False)
    rsi = nc.declare_dram_parameter("rsi", [ROWS, 3], F32, isOutput=False)
    coefa = nc.declare_dram_parameter("coefa", [128, DEG], F32, isOutput=False)
    coefb = nc.declare_dram_parameter("coefb", [128, DEG], F32, isOutput=False)
    # Shape-bearing tag input: makes each reps-variant a distinct HLO module
    # (the NEFF compile cache keys on module fingerprint, which would
    # otherwise collide across reps since all real I/O shapes match).
    repstag = nc.declare_dram_parameter("repstag", [reps, 1], F32, isOutput=False)
    out = nc.declare_dram_parameter("out", [ROWS, 3], F32, isOutput=True)

    with TileContext(nc) as tc:
        with (
            tc.tile_pool(name="const", bufs=1) as cpool,
            tc.tile_pool(name="work", bufs=2) as wpool,
            tc.tile_pool(name="small", bufs=2) as spool,
        ):
            # Replicated j-coordinates, one [128, N] tile per coordinate.
            J = []
            for c in range(3):
                t = cpool.tile([128, N], F32, name=f"J{c}", tag=f"J{c}")
                nc.gpsimd.dma_start(out=t[:], in_=rsj[c])
                J.append(t)
            cA = cpool.tile([128, DEG], F32, tag="cA")
            nc.gpsimd.dma_start(out=cA[:], in_=coefa[:])
            cB = cpool.tile([128, DEG], F32, tag="cB")
            nc.gpsimd.dma_start(out=cB[:], in_=coefb[:])
            rtag = cpool.tile([1, 1], F32, tag="rtag")
            nc.gpsimd.dma_start(out=rtag[:], in_=repstag[reps - 1:reps, :])
            rsib = []
            for ib in range(NIB):
                t = cpool.tile([128, 3], F32, name=f"rsi{ib}", tag=f"rsi{ib}")
                nc.gpsimd.dma_start(out=t[:], in_=rsi[ib * 128:(ib + 1) * 128, :])
                rsib.append(t)

            for rep_ib in range(reps * NIB):
                ib = rep_ib % NIB
                sums = [spool.tile([128, NJT], F32, name=f"sums{c}", tag=f"sums{c}") for c in range(3)]
                for jt in range(NJT):
                    coef = cA if jt < NJT // 2 else cB
                    jsl = slice(jt * JT, (jt + 1) * JT)
                    # u = J - rs_i  in (-10, 10); minimum-image wrap via binary
                    # comparisons (no fp mod on this walrus; Sign() is unusable
                    # because Sign(0)=0 collapses |m| to 0 for pairs with
                    # u == +-5.0 exactly, which do occur among 50M pairs):
                    #   u1 = u - 10*(u >= 5);  m = u1 + 10*(u1 < -5)
                    # At |u|==5 exactly this yields |m|==5, where the force is
                    # exactly 0, so the (sign-ambiguous) boundary is harmless.
                    # Engine split (HW-measured): ACT instructions carry ~2us
                    # fixed cost each on this part, so ACT is reduced to the
                    # single mandatory Exp; everything else is DVE/GpSimd,
                    # balanced so neither engine dominates.
                    m = []
                    for c in range(3):
                        u = wpool.tile([128, JT], F32, name=f"u{c}", tag=f"u{c}")
                        nc.gpsimd.tensor_scalar(
                            u[:], J[c][:, jsl], rsib[ib][:, c:c + 1], None,
                            AOP.subtract)
                        ca = wpool.tile([128, JT], F32, name=f"ca{c}", tag=f"ca{c}")
                        nc.gpsimd.tensor_scalar(
                            ca[:], u[:], 5.0, 10.0, AOP.is_ge, AOP.mult)
                        E1 = nc.gpsimd if c < 2 else nc.vector
                        u1 = wpool.tile([128, JT], F32, name=f"u1{c}", tag=f"u1{c}")
                        E1.tensor_tensor(u1[:], u[:], ca[:], AOP.subtract)
                        cb = wpool.tile([128, JT], F32, name=f"cb{c}", tag=f"cb{c}")
                        nc.gpsimd.tensor_scalar(
                            cb[:], u1[:], -5.0, 10.0, AOP.is_lt, AOP.mult)
                        E2 = nc.gpsimd if c < 1 else nc.vector
                        mc = wpool.tile([128, JT], F32, name=f"m{c}", tag=f"m{c}")
                        E2.tensor_tensor(mc[:], u1[:], cb[:], AOP.add)
                        m.append(mc)
                    sq = []
                    for c in range(3):
                        s = wpool.tile([128, JT], F32, name=f"sq{c}", tag=f"sq{c}")
                        nc.vector.tensor_tensor(s[:], m[c][:], m[c][:], AOP.mult)
                        sq.append(s)
                    s3 = wpool.tile([128, JT], F32, tag="s3")
                    nc.vector.tensor_tensor(s3[:], sq[0][:], sq[1][:], AOP.add)
                    r2 = wpool.tile([128, JT], F32, tag="r2")
                    nc.vector.tensor_tensor(r2[:], s3[:], sq[2][:], AOP.add)
                    # g = clamp(1 - 0.04*r2, >= GMIN);  v = 1/g exactly on DVE
                    g = wpool.tile([128, JT], F32, tag="g")
                    nc.vector.tensor_scalar(
                        g[:], r2[:], -0.04, 1.0, AOP.mult, AOP.add)
                    gc = wpool.tile([128, JT], F32, tag="gc")
                    nc.vector.tensor_scalar(gc[:], g[:], GMIN, None, AOP.max)
                    v = wpool.tile([128, JT], F32, tag="v")
                    nc.vector.reciprocal(v[:], gc[:])
                    dcy = wpool.tile([128, JT], F32, tag="dcy")
                    nc.scalar.activation(dcy[:], v[:], AF.Exp, bias=1.0,
                                         scale=-1.0)
                    # Horner: F = (((c_D*d + c_{D-1})*d + ...)*d + c_1)*d
                    # via u_k = (u_{k+1} + c_k)*d, u_D = c_D*d; exact since c_0 = 0.
                    acc = wpool.tile([128, JT], F32, tag="acc0")
                    nc.vector.tensor_scalar(
                        acc[:], dcy[:], coef[:, DEG - 1:DEG], None, AOP.mult)
                    for k in range(DEG - 1, 0, -1):
                        nxt = wpool.tile([128, JT], F32, name=f"acc{(DEG - k) % 2}", tag=f"acc{(DEG - k) % 2}")
                        nc.vector.scalar_tensor_tensor(
                            nxt[:], acc[:], coef[:, k - 1:k], dcy[:],
                            AOP.add, AOP.mult)
                        acc = nxt
                    # Row-sums of F*m_c  (accumulated per j-tile into sums[c])
                    for c in range(3):
                        scratch = wpool.tile([128, JT], F32, tag="scratch")
                        nc.vector.scalar_tensor_tensor(
                            scratch[:], acc[:], 0.0, m[c][:],
                            AOP.bypass, AOP.mult,
                            accum_out=sums[c][:, jt:jt + 1])
                # Finalize block: out_rows = rs_i - sum(F*m)   (m = -true disp)
                res = spool.tile([128, 3], F32, tag="res")
                for c in range(3):
                    tot = spool.tile([128, 1], F32, name=f"tot{c}", tag=f"tot{c}")
                    nc.vector.tensor_reduce(
                        tot[:], sums[c][:], mybir.AxisListType.X, AOP.add)
                    nc.vector.tensor_scalar(
                        res[:, c:c + 1], tot[:], rsib[ib][:, c:c + 1], -1.0,
                        AOP.subtract, AOP.mult)
                nc.sync.dma_start(out=out[ib * 128:(ib + 1) * 128, :], in_=res[:])
    return nc


def _split_multi_waits(bir_json: bytes) -> bytes:
    """This walrus build rejects instructions carrying more than one sync
    wait ("Too many sync wait commands").  Hoist all-but-one wait of every
    instruction onto injected same-engine NoOps placed immediately before it
    (same blocking point on that engine's sequencer, so semantics are
    unchanged)."""
    import json as _json
    d = _json.loads(bir_json)
    for fn in d["functions"]:
        for blk in fn["blocks"]:
            new_insts = []
            for inst in blk["instructions"]:
                si = inst.get("sync_info")
                waits = (si or {}).get("on_wait") or []
                if len(waits) > 1:
                    for i, w in enumerate(waits[:-1]):
                        new_insts.append({
                            "debug": inst.get("debug", 0),
                            "engine": inst["engine"],
                            "ins": [],
                            "outs": [],
                            "name": f"{inst['name']}-w{i}",
                            "opcode": "NoOp",
                            "text_hint": "split_wait",
                            "sync_info": {"on_update": [], "on_wait": [w]},
                        })
                    si["on_wait"] = [waits[-1]]
                new_insts.append(inst)
            blk["instructions"] = new_insts
    return _json.dumps(d).encode()


def _get_program(reps=1):
    if reps not in _CACHED:
        nc = _build_program(reps)
        orig = nc.to_json_bytes
        nc.to_json_bytes = lambda: _split_multi_waits(orig())
        _CACHED[reps] = nc
    return _CACHED[reps]


def kernel(rs, same_w1, same_b1, same_wo, same_bo,
           diff_w1, diff_b1, diff_wo, diff_bo):
    global LAST_RESULTS
    rs = np.ascontiguousarray(np.asarray(rs, np.float32))
    coef_same = _fit_poly(same_w1, same_b1, same_wo, same_bo)
    coef_diff = _fit_poly(diff_w1, diff_b1, diff_wo, diff_bo)
    cs = np.ascontiguousarray(np.broadcast_to(coef_same[None, :], (128, DEG)))
    cd = np.ascontiguousarray(np.broadcast_to(coef_diff[None, :], (128, DEG)))

    rsj = np.ascontiguousarray(
        np.broadcast_to(rs.T[:, None, :], (3, 128, N)).astype(np.float32))

    in_maps = []
    for core in range(NCORES):
        up = (core * ROWS) < N_UP  # this core's rows are all one spin block
        in_maps.append({
            "rsj": rsj,
            "rsi": np.ascontiguousarray(rs[core * ROWS:(core + 1) * ROWS, :]),
            "coefa": cs if up else cd,   # coeffs for j < 2048
            "coefb": cd if up else cs,   # coeffs for j >= 2048
            "repstag": np.zeros((1, 1), np.float32),
        })

    nc = _get_program()
    LAST_RESULTS = run_bass_kernel_spmd(nc, in_maps, list(range(NCORES)))
    outs = [np.asarray(LAST_RESULTS.results[i]["out"]) for i in range(NCORES)]
    return np.concatenate(outs, axis=0).astype(np.float32)



# revision 2
# speedup vs baseline: 9.1435x; 9.1435x over previous
"""Trainium2 Bass kernel for the neural-backflow problem — structure-factor
(Fourier) algorithm, fully sharded across 8 cores with a tiny AllReduce.

Math: the pair summand f_c(u) = m_c(u) * F(|m(u)|) (m = minimum-image
displacement, F = decayed-MLP force) is a smooth periodic function on the
torus [0,L)^3 — F vanishes with all derivatives at r = L/2.  Hence

  backflow_c(i) = sum_j f_c(r_i - r_j)
                = sum_{K in half-lattice} 2*gamma_c(K) * (C_i ImS - S_i ReS)

with K = 2*pi*n/L, gamma_c(K) = (K_c/|K|) * Phi'(|K|) / V,
Phi(k) = 4*pi * int_0^{L/2} F(r) r^2 sinc(kr) dr  (1D host quadrature),
S(K) = sum_j e^{i K.r_j} (structure factor), C_i/S_i = cos/sin(K.r_i).
Spin blocks (same/diff MLPs) are handled with separate S_up/S_dn and
per-spin gamma.  NK = 1024 modes (top-|Phi'| from the |n|<=9 ball) gives
rel err ~4e-3 vs the 2e-2 gate (validated host-side in emulation).

Device pipeline per core (own 512 electrons only):
  t[k,j] = n[k] . r_j / L           PE matmul, bf16 hi/lo split (exact)
  fr  = t - rint(t)                 magic-constant round (DVE)
  fr2 = frac-center(t + 1/4)        for cos; derived from fr (Pool+DVE)
  S = sin(2*pi*fr), C = sin(2*pi*fr2)   ACT Sin (accurate on [-pi,pi]),
      with accum_out riding along -> per-core partial ImS/ReS sums
  partial sums spread into up/dn slots by per-core 0/1 selectors,
  [128,32] f32 AllReduce  ->  full ImS/ReS for both spin blocks
  A[k,c] = g1*ImS_up + g2*ImS_dn ; B[k,c] = -(g1*ReS_up + g2*ReS_dn)
  backflow = C_own^T A + S_own^T B  (PE, accumulate in PSUM [3, 512])
  out = backflow + rs_own           DMA out as [3, 512]

g1/g2 fold the factor 2 and the core's spin (same/diff swap) host-side.
"""

import numpy as np
import ml_dtypes

import concourse.bass as bass
import concourse.mybir as mybir
from concourse.tile import TileContext
from concourse.bass_utils import run_bass_kernel_spmd

L = 10.0
N = 4096
N_UP = 2048
NCORES = 8
ROWS = N // NCORES          # 512 electrons per core
NK = 1024                   # half-space K modes
NKT = NK // 128             # 8 k-tiles
NBALL = 9                   # lattice ball radius for mode candidates
MAGIC = float(np.float32(3 * 2 ** 22))
TWO_PI = float(2 * np.pi)

F32 = mybir.dt.float32
BF16 = mybir.dt.bfloat16
BF = ml_dtypes.bfloat16
AOP = mybir.AluOpType
AF = mybir.ActivationFunctionType

LAST_RESULTS = None
_CACHED = {}


# ----------------------------------------------------------------- host math
def _half_lattice(nmax):
    rng = np.arange(-nmax, nmax + 1)
    nx, ny, nz = np.meshgrid(rng, rng, rng, indexing="ij")
    n = np.stack([nx.ravel(), ny.ravel(), nz.ravel()], axis=1)
    n2 = (n ** 2).sum(1)
    keep = (n2 > 0) & (n2 <= nmax * nmax)
    n = n[keep]
    half = (n[:, 0] > 0) | ((n[:, 0] == 0) & (n[:, 1] > 0)) | \
           ((n[:, 0] == 0) & (n[:, 1] == 0) & (n[:, 2] > 0))
    return n[half]


def _F_of_r(r, w1, b1, wo, bo):
    x_cut = L / 2
    xn = np.clip(r / x_cut, 0.0, 1.0 - 1e-05)
    decay = np.exp(1.0 - 1.0 / (1.0 - xn ** 2))
    z = decay[:, None] * np.asarray(w1, np.float64).ravel()[None, :] \
        + np.asarray(b1, np.float64).ravel()[None, :]
    sw = z / (1.0 + np.exp(-z))
    out = sw @ np.asarray(wo, np.float64).ravel() \
        + np.asarray(bo, np.float64).ravel()[0]
    return out * decay


def _phi_prime(ks, wts, nr=20001):
    r = np.linspace(1e-9, L / 2, nr)
    Fr = _F_of_r(r, *wts)
    s = np.outer(ks, r)                       # (nk, nr)
    d_sinc = r[None, :] * (s * np.cos(s) - np.sin(s)) / np.maximum(s * s, 1e-300)
    return 4 * np.pi * np.trapezoid(Fr[None, :] * r[None, :] ** 2 * d_sinc,
                                    r, axis=1)


def _select_modes(weights_same, weights_diff):
    """Top-NK half-space modes by max |Phi'|; returns (n, pp_same, pp_diff)."""
    n = _half_lattice(NBALL)
    k = 2 * np.pi * np.linalg.norm(n, axis=1) / L
    kr = np.round(k, 12)
    ku, inv = np.unique(kr, return_inverse=True)
    pp_s = _phi_prime(ku, weights_same)[inv]
    pp_d = _phi_prime(ku, weights_diff)[inv]
    w = np.maximum(np.abs(pp_s), np.abs(pp_d))
    sel = np.argsort(-w, kind="stable")[:NK]
    return n[sel], pp_s[sel], pp_d[sel]


def _gammas(n, pp):
    K = 2 * np.pi * n / L
    k = np.linalg.norm(K, axis=1)
    V = L ** 3
    return (K / k[:, None]) * pp[:, None] / V          # (NK, 3)


# ------------------------------------------------------------- device program
def _build_program(reps=1, debug=False):
    nc = bass.Bass(num_devices=NCORES)
    wmat = nc.declare_dram_parameter("wmat", [8, NK], BF16, isOutput=False)
    mov = nc.declare_dram_parameter("mov", [8, ROWS], BF16, isOutput=False)
    g1p = nc.declare_dram_parameter("g1p", [NK, 3], F32, isOutput=False)
    g2p = nc.declare_dram_parameter("g2p", [NK, 3], F32, isOutput=False)
    selp = nc.declare_dram_parameter("selp", [128, 2], F32, isOutput=False)
    rsit = nc.declare_dram_parameter("rsit", [3, ROWS], F32, isOutput=False)
    repstag = nc.declare_dram_parameter("repstag", [reps, 1], F32,
                                        isOutput=False)
    out = nc.declare_dram_parameter("out", [3, ROWS], F32, isOutput=True)
    if debug:
        dbg = {
            nm: nc.declare_dram_parameter(nm, shp, F32, isOutput=True)
            for nm, shp in (("dt0", [128, ROWS]), ("dfr0", [128, ROWS]),
                            ("dfr20", [128, ROWS]), ("dS0", [128, ROWS]),
                            ("dC0", [128, ROWS]), ("dmine", [128, 16]),
                            ("dsred", [128, 32]), ("dacoef", [128, 3]),
                            ("dbcoef", [128, 3]), ("dpb", [3, ROWS]))
        }

    with TileContext(nc) as tc:
        with (
            tc.tile_pool(name="const", bufs=1) as cpool,
            tc.tile_pool(name="dram", bufs=2, space="DRAM") as dpool,
            tc.tile_pool(name="work", bufs=3) as wpool,
            tc.tile_pool(name="own", bufs=2) as opool,
            tc.tile_pool(name="small", bufs=2) as spool,
            tc.tile_pool(name="psum", bufs=4, space="PSUM") as ppool,
            tc.tile_pool(name="psumb", bufs=2, space="PSUM") as pbpool,
        ):
            wm = cpool.tile([8, NK], BF16, tag="wm")
            nc.gpsimd.dma_start(out=wm[:], in_=wmat[:])
            mv = cpool.tile([8, ROWS], BF16, tag="mv")
            nc.gpsimd.dma_start(out=mv[:], in_=mov[:])
            # g tiles: DRAM [NK,3] viewed as [128 part, NKT, 3]
            g1t = cpool.tile([128, NKT, 3], F32, tag="g1t")
            nc.gpsimd.dma_start(
                out=g1t[:], in_=bass.AP(g1p, 0, [[3, 128], [384, NKT], [1, 3]]))
            g2t = cpool.tile([128, NKT, 3], F32, tag="g2t")
            nc.gpsimd.dma_start(
                out=g2t[:], in_=bass.AP(g2p, 0, [[3, 128], [384, NKT], [1, 3]]))
            sel = cpool.tile([128, 2], F32, tag="sel")
            nc.gpsimd.dma_start(out=sel[:], in_=selp[:])
            rst = cpool.tile([3, ROWS], F32, tag="rst")
            nc.gpsimd.dma_start(out=rst[:], in_=rsit[:])
            rtag = cpool.tile([1, 1], F32, tag="rtag")
            nc.gpsimd.dma_start(out=rtag[:], in_=repstag[reps - 1:reps, :])

            for rep in range(reps):
                Sown = opool.tile([128, NKT, ROWS], BF16, tag="Sown")
                Cown = opool.tile([128, NKT, ROWS], BF16, tag="Cown")
                mine = spool.tile([128, 16], F32, tag="mine")
                for kt in range(NKT):
                    pt = ppool.tile([128, ROWS], F32, tag="pt",
                                    name=f"pt{kt}")
                    nc.tensor.matmul(pt[:], wm[:, kt * 128:(kt + 1) * 128],
                                     mv[:], start=True, stop=True)
                    kk = wpool.tile([128, ROWS], F32, tag="kk")
                    nc.vector.tensor_scalar(kk[:], pt[:], MAGIC, MAGIC,
                                            AOP.add, AOP.subtract)
                    fr = wpool.tile([128, ROWS], F32, tag="fr")
                    nc.vector.tensor_tensor(fr[:], pt[:], kk[:], AOP.subtract)
                    c1 = wpool.tile([128, ROWS], F32, tag="c1")
                    nc.gpsimd.tensor_scalar(c1[:], fr[:], 0.25, -1.0,
                                            AOP.is_ge, AOP.mult)
                    fr2 = wpool.tile([128, ROWS], F32, tag="fr2")
                    nc.vector.scalar_tensor_tensor(fr2[:], c1[:], 0.25, fr[:],
                                                   AOP.add, AOP.add)
                    nc.scalar.activation(Sown[:, kt, :], fr[:], AF.Sin,
                                         scale=TWO_PI,
                                         accum_out=mine[:, kt:kt + 1])
                    nc.scalar.activation(Cown[:, kt, :], fr2[:], AF.Sin,
                                         scale=TWO_PI,
                                         accum_out=mine[:, 8 + kt:9 + kt])
                    if debug and rep == 0 and kt == 0:
                        dcp = wpool.tile([128, ROWS], F32, tag="dcp")
                        nc.vector.tensor_copy(dcp[:], pt[:])
                        nc.sync.dma_start(out=dbg["dt0"][:], in_=dcp[:])
                        nc.sync.dma_start(out=dbg["dfr0"][:], in_=fr[:])
                        nc.sync.dma_start(out=dbg["dfr20"][:], in_=fr2[:])
                        dS = wpool.tile([128, ROWS], F32, tag="dS")
                        nc.vector.tensor_copy(dS[:], Sown[:, kt, :])
                        nc.sync.dma_start(out=dbg["dS0"][:], in_=dS[:])
                        dC = wpool.tile([128, ROWS], F32, tag="dC")
                        nc.vector.tensor_copy(dC[:], Cown[:, kt, :])
                        nc.sync.dma_start(out=dbg["dC0"][:], in_=dC[:])
                # place partial sums into up/dn slots by spin selector
                redin = spool.tile([128, 32], F32, tag="redin")
                nc.gpsimd.tensor_scalar(redin[:, 0:8], mine[:, 0:8],
                                        sel[:, 0:1], None, AOP.mult)
                nc.gpsimd.tensor_scalar(redin[:, 8:16], mine[:, 0:8],
                                        sel[:, 1:2], None, AOP.mult)
                nc.gpsimd.tensor_scalar(redin[:, 16:24], mine[:, 8:16],
                                        sel[:, 0:1], None, AOP.mult)
                nc.gpsimd.tensor_scalar(redin[:, 24:32], mine[:, 8:16],
                                        sel[:, 1:2], None, AOP.mult)
                ib = dpool.tile([128, 32], F32, name="ib")
                ob = dpool.tile([128, 32], F32, name="ob")
                nc.sync.dma_start(out=ib[:], in_=redin[:])
                nc.gpsimd.collective_compute(
                    "AllReduce", AOP.add,
                    replica_groups=[list(range(NCORES))],
                    ins=[ib.opt()], outs=[ob.opt()])
                sred = spool.tile([128, 32], F32, tag="sred")
                nc.gpsimd.dma_start(out=sred[:], in_=ob[:])
                # negate Re columns (16..31)
                nre = spool.tile([128, 16], F32, tag="nre")
                nc.gpsimd.tensor_scalar(nre[:], sred[:, 16:32], -1.0, None,
                                        AOP.mult)
                # coefficient combine: A = g1*ImS_up + g2*ImS_dn,
                #                      B = g1*(-ReS_up) + g2*(-ReS_dn)
                acoef = spool.tile([128, NKT, 3], BF16, tag="acoef")
                bcoef = spool.tile([128, NKT, 3], BF16, tag="bcoef")
                for kt in range(NKT):
                    tA = spool.tile([128, 3], F32, tag="tA", name=f"tA{kt}")
                    nc.gpsimd.tensor_scalar(tA[:], g1t[:, kt, :],
                                            sred[:, kt:kt + 1], None, AOP.mult)
                    nc.vector.scalar_tensor_tensor(
                        acoef[:, kt, :], g2t[:, kt, :], sred[:, 8 + kt:9 + kt],
                        tA[:], AOP.mult, AOP.add)
                    tB = spool.tile([128, 3], F32, tag="tB", name=f"tB{kt}")
                    nc.gpsimd.tensor_scalar(tB[:], g1t[:, kt, :],
                                            nre[:, kt:kt + 1], None, AOP.mult)
                    nc.vector.scalar_tensor_tensor(
                        bcoef[:, kt, :], g2t[:, kt, :], nre[:, 8 + kt:9 + kt],
                        tB[:], AOP.mult, AOP.add)
                # phase B: backflow[3, ROWS] = sum_k C^T A + S^T B
                # accumulation groups must be consecutive per PSUM region
                pb = pbpool.tile([3, ROWS], F32, tag="pb")
                for ic in range(ROWS // 128):
                    for kt in range(NKT):
                        for part, (coef, ownt) in enumerate(
                                ((acoef, Cown), (bcoef, Sown))):
                            nc.tensor.matmul(
                                pb[:, ic * 128:(ic + 1) * 128],
                                coef[:, kt, :],
                                ownt[:, kt, ic * 128:(ic + 1) * 128],
                                start=(kt == 0 and part == 0),
                                stop=(kt == NKT - 1 and part == 1))
                outT = spool.tile([3, ROWS], F32, tag="outT")
                nc.vector.tensor_tensor(outT[:], pb[:], rst[:], AOP.add)
                nc.sync.dma_start(out=out[:], in_=outT[:])
                if debug and rep == 0:
                    nc.sync.dma_start(out=dbg["dmine"][:], in_=mine[:])
                    nc.sync.dma_start(out=dbg["dsred"][:], in_=sred[:])
                    dac = spool.tile([128, 3], F32, tag="dac")
                    nc.vector.tensor_copy(dac[:], acoef[:, 0, :])
                    nc.sync.dma_start(out=dbg["dacoef"][:], in_=dac[:])
                    dbc = spool.tile([128, 3], F32, tag="dbc")
                    nc.vector.tensor_copy(dbc[:], bcoef[:, 0, :])
                    nc.sync.dma_start(out=dbg["dbcoef"][:], in_=dbc[:])
                    dpbt = spool.tile([3, ROWS], F32, tag="dpbt")
                    nc.vector.tensor_copy(dpbt[:], pb[:])
                    nc.sync.dma_start(out=dbg["dpb"][:], in_=dpbt[:])
    return nc


def _split_multi_waits(bir_json: bytes) -> bytes:
    """This walrus build rejects instructions carrying more than one sync
    wait ("Too many sync wait commands").  Hoist all-but-one wait of every
    instruction onto injected same-engine NoOps placed immediately before it
    (same blocking point on that engine's sequencer, so semantics are
    unchanged)."""
    import json as _json
    d = _json.loads(bir_json)
    for fn in d["functions"]:
        for blk in fn["blocks"]:
            new_insts = []
            for inst in blk["instructions"]:
                si = inst.get("sync_info")
                waits = (si or {}).get("on_wait") or []
                if len(waits) > 1:
                    for i, w in enumerate(waits[:-1]):
                        new_insts.append({
                            "debug": inst.get("debug", 0),
                            "engine": inst["engine"],
                            "ins": [],
                            "outs": [],
                            "name": f"{inst['name']}-w{i}",
                            "opcode": "NoOp",
                            "text_hint": "split_wait",
                            "sync_info": {"on_update": [], "on_wait": [w]},
                        })
                    si["on_wait"] = [waits[-1]]
                new_insts.append(inst)
            blk["instructions"] = new_insts
    return _json.dumps(d).encode()


def _get_program(reps=1):
    if reps not in _CACHED:
        nc = _build_program(reps)
        orig = nc.to_json_bytes
        nc.to_json_bytes = lambda: _split_multi_waits(orig())
        _CACHED[reps] = nc
    return _CACHED[reps]


# ---------------------------------------------------------------- host driver
def prepare_in_maps(rs, same_w1, same_b1, same_wo, same_bo,
                    diff_w1, diff_b1, diff_wo, diff_bo, reps=1):
    rs = np.ascontiguousarray(np.asarray(rs, np.float32))
    n, pp_s, pp_d = _select_modes(
        (same_w1, same_b1, same_wo, same_bo),
        (diff_w1, diff_b1, diff_wo, diff_bo))
    g_same = (2.0 * _gammas(n, pp_s)).astype(np.float32)   # (NK, 3)
    g_diff = (2.0 * _gammas(n, pp_d)).astype(np.float32)

    x = (rs / np.float32(L)).astype(np.float32)            # (N, 3) in [0,1)
    a16 = x.astype(BF)
    b16 = (x - a16.astype(np.float32)).astype(BF)

    wmat = np.zeros((8, NK), BF)
    wmat[0:3, :] = n.T.astype(BF)
    wmat[3:6, :] = n.T.astype(BF)

    in_maps = []
    for core in range(NCORES):
        up = (core * ROWS) < N_UP
        sl = slice(core * ROWS, (core + 1) * ROWS)
        mov = np.zeros((8, ROWS), BF)
        mov[0:3, :] = a16[sl].T
        mov[3:6, :] = b16[sl].T
        selv = np.zeros((128, 2), np.float32)
        selv[:, 0 if up else 1] = 1.0
        in_maps.append({
            "wmat": wmat,
            "mov": mov,
            "g1p": np.ascontiguousarray(g_same if up else g_diff),
            "g2p": np.ascontiguousarray(g_diff if up else g_same),
            "selp": selv,
            "rsit": np.ascontiguousarray(rs[sl].T),
            "repstag": np.zeros((reps, 1), np.float32),
        })
    return in_maps


def kernel(rs, same_w1, same_b1, same_wo, same_bo,
           diff_w1, diff_b1, diff_wo, diff_bo):
    global LAST_RESULTS
    in_maps = prepare_in_maps(rs, same_w1, same_b1, same_wo, same_bo,
                              diff_w1, diff_b1, diff_wo, diff_bo)
    nc = _get_program()
    LAST_RESULTS = run_bass_kernel_spmd(nc, in_maps, list(range(NCORES)))
    outs = [np.asarray(LAST_RESULTS.results[i]["out"]).T
            for i in range(NCORES)]
    return np.ascontiguousarray(
        np.concatenate(outs, axis=0).astype(np.float32))
